# revision 3
# baseline (speedup 1.0000x reference)
"""Block-sparse attention on 8 Trainium2 NeuronCores (Bass/Tile SPMD kernel).

Sharding: batch*head_groups across the 8 cores. Core c handles batch c//4 and
heads [4*(c%4), 4*(c%4)+4). Projection weights are sliced per core host-side
(pre-transposed + bf16-cast); the [16,16] block mask specializes the compiled
program (only kept blocks are computed). Each core emits a partial output
(its 256-wide d-slice pushed through Wo); the host sums the 4 partials per
batch and adds the bias.

Layout strategy per core (all attention math in "transposed" orientation):
  - x^T [1024, 2048] bf16 resident in SBUF (8 partition tiles)
  - q^T, k^T computed as [256, 2048] (2 pair-tiles of 128 partitions: each
    pair-tile stacks 2 heads of 64 rows), v natural [2048, 256]
  - scores^T block (j,i) = kT_j.T @ qT_i -> PSUM [128 k, 128 q]; two heads
    row-packed (K=64 at base partitions 0/64) run concurrently on the PE
  - exp on ScalarE in [128, <=1024] groups, PSUM -> SBUF bf16
  - out2^T = v.T @ attn^T col-packed (M=64 at out partitions 0/64), plus M=1
    ones-matmuls producing the softmax denominators in the same PSUM bank
  - normalize via reciprocal + DMA partition-broadcast + one DVE multiply
  - final: out_partial[s, :] accumulates outTbf[p].T @ woT[p] over 2 pairs
"""

import time
from contextlib import ExitStack

import ml_dtypes
import numpy as np

import concourse.bass as bass
import concourse.tile as tile
from concourse import bacc, mybir
from concourse.ap import AP as APClass
from concourse.bass_utils import run_bass_kernel_spmd

BF16 = mybir.dt.bfloat16
F32 = mybir.dt.float32
bf16 = ml_dtypes.bfloat16

B, S, D, H = 2, 2048, 1024, 16
DH = 64
BLK = 128
NB = 16
NCORES = 8
HPC = H // (NCORES // B)   # 4 heads per core
E = HPC * DH               # 256 projection columns per core
KD = D // 128              # 8 contraction chunks
EXP_GROUP = 8              # k-blocks per exp call ([128, 1024] = 2 PSUM banks)

_nc_cache: dict = {}
last_run_info: dict = {}


def _bcast_ap(sl, n):
    """[1, W] SBUF slice -> [1, n, W] AP replicating the row n times (for DMA)."""
    apl = [list(x) for x in sl.ap]
    assert len(apl) == 2 and apl[0][1] == 1, apl
    return APClass(sl.tensor, sl.offset, [apl[0], [0, n], apl[1]])


def _emit(tc, aps, kept):
    nc = tc.nc
    xT_ap, wqT_ap, wkT_ap, wvT_ap, woT_ap, outp_ap = aps
    Exp = mybir.ActivationFunctionType.Exp

    with ExitStack() as ctx:
        const = ctx.enter_context(tc.tile_pool(name="const", bufs=1))
        persist = ctx.enter_context(tc.tile_pool(name="persist", bufs=1))

        # ---- Phase 0: load inputs -------------------------------------------------
        xT = []
        for kd in range(KD):
            t = persist.tile([128, S], BF16, name=f"xT{kd}", tag=f"xT{kd}")
            nc.sync.dma_start(t[:], xT_ap[kd * 128:(kd + 1) * 128, :])
            xT.append(t)

        def load_w(src_ap, name):
            ts = []
            for kd in range(KD):
                t = persist.tile([128, E], BF16, name=f"{name}{kd}", tag=f"{name}{kd}")
                nc.sync.dma_start(t[:], src_ap[kd * 128:(kd + 1) * 128, :])
                ts.append(t)
            return ts

        wq = load_w(wqT_ap, "wq")
        wk = load_w(wkT_ap, "wk")
        wv = load_w(wvT_ap, "wv")
        wo = []
        for p in range(2):
            t = persist.tile([128, D], BF16, name=f"wo{p}", tag=f"wo{p}")
            nc.sync.dma_start(t[:], woT_ap[p * 128:(p + 1) * 128, :])
            wo.append(t)

        ones = const.tile([128, 1], BF16)
        nc.any.memset(ones[:], 1.0)

        qT = [persist.tile([128, S], BF16, name=f"qT{p}", tag=f"qT{p}") for p in range(2)]
        kT = [persist.tile([128, S], BF16, name=f"kT{p}", tag=f"kT{p}") for p in range(2)]
        vv = [persist.tile([128, E], BF16, name=f"v{m}", tag=f"v{m}") for m in range(S // 128)]
        outTbf = [persist.tile([128, S], BF16, name=f"oT{p}", tag=f"oT{p}") for p in range(2)]

        # ---- Phase 1: projections -------------------------------------------------
        with tc.tile_pool(name="proj_ps", bufs=8, space="PSUM") as proj_ps:
            # q^T and k^T: stationary = weight chunk, moving = x^T s-chunks
            for dst, w in ((qT, wq), (kT, wk)):
                for p in range(2):
                    pss = [proj_ps.tile([128, 512], F32, name="projps", tag="proj") for _ in range(4)]
                    for kd in range(KD):
                        for sc in range(4):
                            nc.tensor.matmul(
                                pss[sc][:],
                                w[kd][:, p * 128:(p + 1) * 128],
                                xT[kd][:, sc * 512:(sc + 1) * 512],
                                start=(kd == 0),
                                stop=(kd == KD - 1),
                            )
                    for sc in range(4):
                        nc.vector.tensor_copy(dst[p][:, sc * 512:(sc + 1) * 512], pss[sc][:])
            # v natural: stationary = x^T s-tile chunk, moving = wv
            for m in range(S // 128):
                ps = proj_ps.tile([128, 512], F32, name="projv", tag="proj")
                for kd in range(KD):
                    nc.tensor.matmul(
                        ps[:, 0:E],
                        xT[kd][:, m * 128:(m + 1) * 128],
                        wv[kd][:],
                        start=(kd == 0),
                        stop=(kd == KD - 1),
                    )
                nc.vector.tensor_copy(vv[m][:], ps[:, 0:E])

        # ---- Phase 2: block-sparse attention -------------------------------------
        with ExitStack() as actx:
            sc_pool = actx.enter_context(tc.tile_pool(name="sc_ps", bufs=3, space="PSUM"))
            av_pool = actx.enter_context(tc.tile_pool(name="av_ps", bufs=2, space="PSUM"))
            attn_pool = actx.enter_context(tc.tile_pool(name="attn_sb", bufs=10))
            norm_pool = actx.enter_context(tc.tile_pool(name="norm_sb", bufs=6))

            def emit_scores(p, i):
                js = kept[i]
                groups = []
                for g in range(0, len(js), EXP_GROUP):
                    gjs = js[g:g + EXP_GROUP]
                    w_cols = len(gjs) * 128
                    psA = sc_pool.tile([128, EXP_GROUP * 128], F32, name="scA", tag="sc")
                    psB = sc_pool.tile([128, EXP_GROUP * 128], F32, name="scB", tag="sc")
                    for idx, j in enumerate(gjs):
                        off = idx * 128
                        nc.tensor.matmul(
                            psA[:, off:off + 128],
                            kT[p][0:64, j * 128:(j + 1) * 128],
                            qT[p][0:64, i * 128:(i + 1) * 128],
                        )
                        nc.tensor.matmul(
                            psB[:, off:off + 128],
                            kT[p][64:128, j * 128:(j + 1) * 128],
                            qT[p][64:128, i * 128:(i + 1) * 128],
                        )
                    aA = attn_pool.tile([128, EXP_GROUP * 128], BF16, name="aA", tag="attn")
                    aB = attn_pool.tile([128, EXP_GROUP * 128], BF16, name="aB", tag="attn")
                    nc.scalar.activation(aA[:, 0:w_cols], psA[:, 0:w_cols], Exp)
                    nc.scalar.activation(aB[:, 0:w_cols], psB[:, 0:w_cols], Exp)
                    groups.append((gjs, aA, aB))
                return groups

            def emit_av_norm(p, i, groups):
                js = kept[i]
                njs = len(js)
                avp = av_pool.tile([128, 256], F32, name="avp", tag="av")
                cnt = 0
                for gjs, aA, aB in groups:
                    for idx, j in enumerate(gjs):
                        off = idx * 128
                        st, sp = cnt == 0, cnt == njs - 1
                        nc.tensor.matmul(
                            avp[0:64, 0:128],
                            vv[j][:, (2 * p) * 64:(2 * p) * 64 + 64],
                            aA[:, off:off + 128],
                            start=st, stop=sp,
                        )
                        nc.tensor.matmul(
                            avp[64:128, 0:128],
                            vv[j][:, (2 * p + 1) * 64:(2 * p + 1) * 64 + 64],
                            aB[:, off:off + 128],
                            start=st, stop=sp,
                        )
                        cnt += 1
                cnt = 0
                for gjs, aA, aB in groups:
                    for idx, j in enumerate(gjs):
                        off = idx * 128
                        st, sp = cnt == 0, cnt == njs - 1
                        nc.tensor.matmul(
                            avp[0:1, 128:256], ones[:], aA[:, off:off + 128],
                            start=st, stop=sp,
                        )
                        nc.tensor.matmul(
                            avp[32:33, 128:256], ones[:], aB[:, off:off + 128],
                            start=st, stop=sp,
                        )
                        cnt += 1
                # normalization
                icols = slice(i * 128, (i + 1) * 128)
                avstage = norm_pool.tile([128, 128], F32, name="avstage", tag="avst")
                nc.vector.tensor_copy(avstage[:], avp[:, 0:128])
                dn = norm_pool.tile([33, 128], F32, name="dn", tag="dn")
                nc.vector.tensor_copy(dn[0:1, :], avp[0:1, 128:256])
                nc.vector.tensor_copy(dn[32:33, :], avp[32:33, 128:256])
                rc = norm_pool.tile([33, 128], F32, name="rc", tag="rc")
                nc.vector.reciprocal(rc[0:1, :], dn[0:1, :])
                nc.vector.reciprocal(rc[32:33, :], dn[32:33, :])
                bc = norm_pool.tile([128, 128], F32, name="bc", tag="bc")
                nc.sync.dma_start(bc[0:64, :], _bcast_ap(rc[0:1, :], 64))
                nc.sync.dma_start(bc[64:128, :], _bcast_ap(rc[32:33, :], 64))
                nc.vector.tensor_mul(outTbf[p][:, icols], avstage[:], bc[:])

            # 1-deep software pipeline: scores(i) | av+norm(i-1)
            for p in range(2):
                prev = None
                for i in range(NB):
                    groups = emit_scores(p, i)
                    if prev is not None:
                        emit_av_norm(p, prev[0], prev[1])
                    prev = (i, groups)
                emit_av_norm(p, prev[0], prev[1])

        # ---- Phase 3: output projection (partial over this core's d-slice) -------
        with ExitStack() as fctx:
            fin_ps = fctx.enter_context(tc.tile_pool(name="fin_ps", bufs=4, space="PSUM"))
            fin_sb = fctx.enter_context(tc.tile_pool(name="fin_sb", bufs=4))
            for m in range(S // 128):
                pss = [fin_ps.tile([128, 512], F32, name="finps", tag="fin") for _ in range(2)]
                for p in range(2):
                    for n in range(2):
                        nc.tensor.matmul(
                            pss[n][:],
                            outTbf[p][:, m * 128:(m + 1) * 128],
                            wo[p][:, n * 512:(n + 1) * 512],
                            start=(p == 0),
                            stop=(p == 1),
                        )
                for n in range(2):
                    st = fin_sb.tile([128, 512], F32, name="finst", tag="finsb")
                    nc.scalar.copy(st[:], pss[n][:])
                    nc.sync.dma_start(
                        outp_ap[m * 128:(m + 1) * 128, n * 512:(n + 1) * 512], st[:]
                    )


def _get_nc(kept):
    key = kept
    if key in _nc_cache:
        return _nc_cache[key]
    nc = bacc.Bacc("TRN2", target_bir_lowering=False, debug=False, num_devices=NCORES)
    xT_ap = nc.dram_tensor("xT", [D, S], BF16, kind="ExternalInput").ap()
    wqT_ap = nc.dram_tensor("wqT", [D, E], BF16, kind="ExternalInput").ap()
    wkT_ap = nc.dram_tensor("wkT", [D, E], BF16, kind="ExternalInput").ap()
    wvT_ap = nc.dram_tensor("wvT", [D, E], BF16, kind="ExternalInput").ap()
    woT_ap = nc.dram_tensor("woT", [E, D], BF16, kind="ExternalInput").ap()
    outp_ap = nc.dram_tensor("outp", [S, D], F32, kind="ExternalOutput").ap()
    with tile.TileContext(nc) as tc:
        _emit(tc, (xT_ap, wqT_ap, wkT_ap, wvT_ap, woT_ap, outp_ap), kept)
    nc.compile()
    _nc_cache[key] = nc
    return nc


def kernel(x, Wq, Wk, Wv, Wo, bo, block_mask):
    x = np.asarray(x, dtype=np.float32)
    Wq = np.asarray(Wq, dtype=np.float32)
    Wk = np.asarray(Wk, dtype=np.float32)
    Wv = np.asarray(Wv, dtype=np.float32)
    Wo = np.asarray(Wo, dtype=np.float32)
    bo = np.asarray(bo, dtype=np.float32)
    mask = np.asarray(block_mask).astype(bool)

    kept = tuple(tuple(int(j) for j in np.nonzero(mask[i])[0]) for i in range(NB))
    assert all(len(js) > 0 for js in kept), "a query block row has no kept blocks"

    t0 = time.monotonic()
    nc = _get_nc(kept)
    t_compile = time.monotonic() - t0

    xT_b = [np.ascontiguousarray(x[b].T).astype(bf16) for b in range(B)]
    in_maps = []
    for c in range(NCORES):
        b = c // (NCORES // B)
        hs = c % (NCORES // B)
        sl = slice(hs * E, (hs + 1) * E)
        in_maps.append({
            "xT": xT_b[b],
            "wqT": np.ascontiguousarray((Wq[sl, :] / np.sqrt(np.float32(DH))).T).astype(bf16),
            "wkT": np.ascontiguousarray(Wk[sl, :].T).astype(bf16),
            "wvT": np.ascontiguousarray(Wv[sl, :].T).astype(bf16),
            "woT": np.ascontiguousarray(Wo[:, sl].T).astype(bf16),
        })

    t0 = time.monotonic()
    res = run_bass_kernel_spmd(nc, in_maps, list(range(NCORES)))
    t_run = time.monotonic() - t0

    out = np.zeros((B, S, D), np.float32)
    for c in range(NCORES):
        out[c // (NCORES // B)] += res.results[c]["outp"]
    out += bo[None, None, :]

    last_run_info.update(compile_s=t_compile, run_s=t_run, nc=nc)
    return out
